# revision 1
# baseline (speedup 1.0000x reference)
"""Trainium2 Bass kernel: fused embedding gather-concat.

out[r] = concat(proc_pos[process_ids[r]], locs_sp[location_ids[r]])   r in [0, 8M)

Sharding: rows data-parallel across 8 NeuronCores (1M rows each, padded to a
tile multiple); lookup tables replicated in every core's DRAM.

Strategy (vs. the 22.4ms per-128-row indirect-DMA baseline): amortize the
~1us fixed SWDGE descriptor-gen cost over 8192-index InstDMAGatherAnt
gathers (0.46ns/row instead of 8.1ns/row).

  - locs_sp is repacked on host into [31250, 64] f32: 16 table rows of
    16B (xyz + pad) per 256B block. dma_gather with idx = lid>>4 (fits the
    ucode's int16 index) pulls each row's 256B block; a 16-way
    tensor_tensor(is_equal vs iota) mask + copy_predicated chain on DVE
    extracts the right 12B into the assembled out tile.
  - proc_pos is repacked into [16, 64] f32 (256B row stride). A raw
    InstDMAGatherAnt with elem_size=8 f32 / elem_step=64 (the ucode only
    requires the SOURCE STRIDE to be a 256B multiple; bass's elem%256
    assert is transpose-only in HW) gathers the exact 32B row per output
    row, idx = pid. ACT copies it into out[:, 0:8].
  - single_packet=False is required for >1.5k-index gathers (single-packet
    ring bookkeeping overflows the descriptor carveout).
  - Index lists are host-prepared in the ucode's wrapped layout (idx j at
    partition j%16, column j//16, replicated to all 8 16-partition groups
    so every SWDGE queue's q7 pair finds them), permuted so gather dst
    position [j%128, j//128] lands output row p*S+s contiguously for the
    tile store.

Per-tile pipeline (depth-3 slot ring): SP loads idx tiles + stores out
tiles, gpsimd issues the two gathers (rotating SWDGE queues), DVE builds
the 16 offset masks (one tensor_tensor vs iota + drain — back-to-back
dependent DVE ops pipeline-race without it) and runs 16 copy_predicated,
ACT interleaves the proc part.
"""

from contextlib import ExitStack

import numpy as np

import concourse.bass as bass
import concourse.mybir as mybir
from concourse import library_config
from concourse.bass_utils import run_bass_kernel_spmd

N_CORES = 8
NAUG = 8_000_000
PER_CORE = NAUG // N_CORES  # 1,000,000

NUM_PROCS = 16
PROC_DIM = 8
NUM_LOCS = 500_000
SPATIAL_DIM = 3
OUT_DIM = PROC_DIM + SPATIAL_DIM  # 11

NBLK = NUM_LOCS // 16  # 31250 blocks of 16 padded rows each

T = 8192  # rows per tile (= indices per dma_gather)
S = T // 128  # slots per partition
NT = -(-PER_CORE // T)  # 123
N_PAD = NT * T  # 1,007,616
DEPTH = 3


def dma_gather_raw(gp, out_ap, in_ap, idxs_ap, num_idxs, elem_size, elem_step=None,
                   queue_num=0, single_packet=False, num_idxs_reg=None):
    """bass.dma_gather minus the elem_size%256 assert (ucode only requires the
    SOURCE STRIDE to be a multiple of 256B for non-transpose mode)."""
    assert idxs_ap.dtype == mybir.dt.int16
    if elem_step is None:
        elem_step = elem_size
    dt_size = mybir.dt.size(in_ap.dtype)
    stride_bytes = elem_step * dt_size
    assert stride_bytes % 256 == 0 and stride_bytes // 256 < 256
    assert in_ap.ap[0][0] == elem_step
    assert in_ap.ap[-1][1] == elem_size
    assert out_ap.ap[0][1] * out_ap.ap[1][1] == ((num_idxs + 127) // 128) * 128
    assert out_ap.ap[-1][1] == elem_size
    _in_ap = gp.lower_ap_dma(in_ap, for_custom_bir_dma=True)
    _idxs_ap = gp.lower_ap(idxs_ap)
    _out_ap = gp.lower_ap(out_ap)
    return gp.add_instruction(
        mybir.InstDMAGatherAnt(
            name=gp.bass.get_next_instruction_name(),
            ins=[*_in_ap, _idxs_ap, gp.lower_val_access(
                gp.to_reg(num_idxs if num_idxs_reg is None else num_idxs_reg))],
            outs=[_out_ap],
            transpose=False,
            num_idxs=num_idxs,
            elem_size=elem_size,
            stride_bytes_256=stride_bytes // 256,
            gen_mode=0,
            single_packet=single_packet,
            queue_num=queue_num,
            sbuf_tokens_per_rank=0,
            sbuf_free_dim_per_rank=0,
            sbuf_free_dim_pad_per_rank=0,
            sbuf_byte_offset=0,
        )
    )


def build_nc():
    nc = bass.Bass(num_swdge_queues=4)
    loc64 = nc.declare_dram_parameter("loc64", [NBLK, 64], mybir.dt.float32, isOutput=False)
    proc256 = nc.declare_dram_parameter("proc256", [16, 64], mybir.dt.float32, isOutput=False)
    iota16 = nc.declare_dram_parameter("iota16", [128, 16], mybir.dt.int32, isOutput=False)
    eidx = nc.declare_dram_parameter("eidx", [NT, 128, T // 16], mybir.dt.int16, isOutput=False)
    pidx = nc.declare_dram_parameter("pidx", [NT, 128, T // 16], mybir.dt.int16, isOutput=False)
    off = nc.declare_dram_parameter("off", [NT, 128, S], mybir.dt.int32, isOutput=False)
    out = nc.declare_dram_parameter("out", [N_PAD, OUT_DIM], mybir.dt.float32, isOutput=True)
    out_v = out.rearrange("(b p s) d -> b p (s d)", b=NT, p=128)

    W = T // 16  # wrapped idx columns

    def uses(s):
        return (NT - s + DEPTH - 1) // DEPTH

    with ExitStack() as ctx:
        ldi = ctx.enter_context(nc.semaphore("ldi"))
        ld = [ctx.enter_context(nc.semaphore(f"ld{s}")) for s in range(DEPTH)]
        g = [ctx.enter_context(nc.semaphore(f"g{s}")) for s in range(DEPTH)]
        v = [ctx.enter_context(nc.semaphore(f"v{s}")) for s in range(DEPTH)]
        st = [ctx.enter_context(nc.semaphore(f"st{s}")) for s in range(DEPTH)]
        eidx_sb = ctx.enter_context(nc.sbuf_tensor("eidx_sb", [128, DEPTH * W], mybir.dt.int16))
        pidx_sb = ctx.enter_context(nc.sbuf_tensor("pidx_sb", [128, DEPTH * W], mybir.dt.int16))
        off_sb = ctx.enter_context(nc.sbuf_tensor("off_sb", [128, DEPTH * S], mybir.dt.int32))
        iota_sb = ctx.enter_context(nc.sbuf_tensor("iota_sb", [128, 16], mybir.dt.int32))
        gloc = ctx.enter_context(nc.sbuf_tensor("gloc", [128, DEPTH * S * 64], mybir.dt.float32))
        gproc = ctx.enter_context(nc.sbuf_tensor("gproc", [128, DEPTH * S * 8], mybir.dt.float32))
        outb = ctx.enter_context(nc.sbuf_tensor("outb", [128, DEPTH * S * OUT_DIM], mybir.dt.float32))
        mbuf = ctx.enter_context(nc.sbuf_tensor("mbuf", [128, DEPTH * S * 16], mybir.dt.uint8))
        block = ctx.enter_context(nc.Block())

        def eslot(s):
            return eidx_sb[:, s * W:(s + 1) * W]

        def pslot(s):
            return pidx_sb[:, s * W:(s + 1) * W]

        def oslot(s):
            return off_sb[:, s * S:(s + 1) * S]

        def glslot(s):
            return gloc[:, s * S * 64:(s + 1) * S * 64].rearrange("p (s e) -> p s e", e=64)

        def gpslot(s):
            return gproc[:, s * S * 8:(s + 1) * S * 8].rearrange("p (s e) -> p s e", e=8)

        def oslot_out(s):
            return outb[:, s * S * OUT_DIM:(s + 1) * S * OUT_DIM].rearrange(
                "p (s e) -> p s e", e=OUT_DIM)

        def mslot(s):
            return mbuf[:, s * S * 16:(s + 1) * S * 16].rearrange("p (s e) -> p s e", e=16)

        def issue_loads(sp, b):
            s = b % DEPTH
            sp.dma_start(out=eslot(s), in_=eidx[b]).then_inc(ld[s], 16)
            sp.dma_start(out=pslot(s), in_=pidx[b]).then_inc(ld[s], 16)
            sp.dma_start(out=oslot(s), in_=off[b]).then_inc(ld[s], 16)

        @block.sync
        def _(sp):
            sp.dma_start(out=iota_sb[:], in_=iota16[:]).then_inc(ldi, 16)
            for b in range(min(DEPTH, NT)):
                issue_loads(sp, b)
            for b in range(NT):
                s, u = b % DEPTH, b // DEPTH
                sp.wait_ge(v[s], 2 * (u + 1))
                sp.dma_start(
                    out=out_v[b],
                    in_=outb[:, s * S * OUT_DIM:(s + 1) * S * OUT_DIM],
                ).then_inc(st[s], 16)
                if b + DEPTH < NT:
                    # slot reuse: idx buffers free once this slot's gathers ran
                    sp.wait_ge(g[s], 32 * (u + 1))
                    issue_loads(sp, b + DEPTH)
            for s in range(DEPTH):
                sp.wait_ge(st[s], 16 * uses(s))

        @block.gpsimd
        def _(gp):
            gp.load_library(library_config.mlp)
            t_reg = gp.to_reg(T)
            for b in range(NT):
                s, u = b % DEPTH, b // DEPTH
                gp.wait_ge(ld[s], 48 * (u + 1))
                if u >= 1:
                    # gather dst slots free once the slot's extract ops ran
                    gp.wait_ge(v[s], 2 * u)
                q = b % 4
                gp.dma_gather(
                    glslot(s), loc64[:], eslot(s), T, t_reg, 64,
                    queue_num=q, single_packet=False,
                ).then_inc(g[s], 16)
                dma_gather_raw(
                    gp, gpslot(s), proc256[:, 0:PROC_DIM], pslot(s), T, PROC_DIM,
                    elem_step=64, queue_num=(q + 1) % 4, single_packet=False,
                    num_idxs_reg=t_reg,
                ).then_inc(g[s], 16)
            for s in range(DEPTH):
                gp.wait_ge(g[s], 32 * uses(s))

        @block.vector
        def _(dve):
            dve.wait_ge(ldi, 16)
            for b in range(NT):
                s, u = b % DEPTH, b // DEPTH
                dve.wait_ge(ld[s], 48 * (u + 1))
                dve.wait_ge(g[s], 32 * (u + 1))
                if u >= 1:
                    dve.wait_ge(st[s], 16 * u)
                ob = oslot_out(s)
                gl = glslot(s)
                m = mslot(s)
                dve.tensor_tensor(
                    out=m,
                    in0=oslot(s)[:, :, None].broadcast_to([128, S, 16]),
                    in1=iota_sb[:, None, :].broadcast_to([128, S, 16]),
                    op=mybir.AluOpType.is_equal,
                )
                dve.drain()
                for o in range(16):
                    ins = dve.copy_predicated(
                        ob[:, :, PROC_DIM:OUT_DIM],
                        m[:, :, o, None].broadcast_to([128, S, 3]),
                        gl[:, :, 4 * o:4 * o + 3],
                    )
                ins.then_inc(v[s], 1)

        @block.scalar
        def _(act):
            for b in range(NT):
                s, u = b % DEPTH, b // DEPTH
                act.wait_ge(g[s], 32 * (u + 1))
                if u >= 1:
                    act.wait_ge(st[s], 16 * u)
                act.copy(out=oslot_out(s)[:, :, 0:PROC_DIM], in_=gpslot(s)).then_inc(v[s], 1)

    from concourse.library_overlay import lower_extended_insts

    lower_extended_insts(nc)
    return nc


_nc_cache = {}

# test.py reads this for exec_time_ns / trace info after a traced run.
_last_results = None


def _get_nc():
    if "nc" not in _nc_cache:
        _nc_cache["nc"] = build_nc()
    return _nc_cache["nc"]


def _prep_indices(vals, dtype):
    """[NT*T] row-major -> [NT, 128, T//16] wrapped+replicated gather lists.

    Tile row r (= p*S + s) must sit at gather list position j = s*128 + p;
    the wrapped layout stores position j at [j%16, j//16], replicated to all
    8 16-partition groups.
    """
    a = vals.reshape(NT, 128, S)  # [b, p, s]
    a = a.transpose(0, 2, 1).reshape(NT, T // 16, 16)  # [b, j//16, j%16]
    a = a.transpose(0, 2, 1)  # [b, 16, T//16]
    return np.broadcast_to(a[:, None, :, :], (NT, 8, 16, T // 16)).reshape(
        NT, 128, T // 16).astype(dtype)


def kernel(proc_pos, locs_sp, process_ids, location_ids):
    global _last_results
    proc_pos = np.ascontiguousarray(np.asarray(proc_pos, dtype=np.float32))
    locs_sp = np.ascontiguousarray(np.asarray(locs_sp, dtype=np.float32))
    pids = np.asarray(process_ids).astype(np.int32, copy=False)
    lids = np.asarray(location_ids).astype(np.int32, copy=False)

    loc_pad = np.zeros((NBLK * 16, 4), np.float32)
    loc_pad[:NUM_LOCS, :SPATIAL_DIM] = locs_sp
    loc64 = loc_pad.reshape(NBLK, 64)
    proc256 = np.zeros((16, 64), np.float32)
    proc256[:NUM_PROCS, :PROC_DIM] = proc_pos
    iota16 = np.tile(np.arange(16, dtype=np.int32), (128, 1))

    nc = _get_nc()
    in_maps = []
    for c in range(N_CORES):
        lo, hi = c * PER_CORE, (c + 1) * PER_CORE
        lid_c = np.zeros(N_PAD, np.int32)
        pid_c = np.zeros(N_PAD, np.int32)
        lid_c[:PER_CORE] = lids[lo:hi]
        pid_c[:PER_CORE] = pids[lo:hi]
        in_maps.append(
            {
                "loc64": loc64,
                "proc256": proc256,
                "iota16": iota16,
                "eidx": _prep_indices(lid_c >> 4, np.int16),
                "pidx": _prep_indices(pid_c, np.int16),
                "off": (lid_c & 15).astype(np.int32).reshape(NT, 128, S),
            }
        )

    res = run_bass_kernel_spmd(nc, in_maps, list(range(N_CORES)))
    _last_results = res
    out = np.concatenate([r["out"][:PER_CORE] for r in res.results], axis=0)
    return out



# revision 3
# speedup vs baseline: 3.5071x; 3.5071x over previous
"""Trainium2 Bass kernel: fused embedding gather-concat.

out[r] = concat(proc_pos[process_ids[r]], locs_sp[location_ids[r]])   r in [0, 8M)

Sharding: rows data-parallel across 8 NeuronCores (1M rows each, padded to a
tile multiple); lookup tables replicated in every core's DRAM.

v2 (vs v1's 9.68ms): trace showed GPSIMD desc-gen is the wall: each
dma_gather generates 8192 SWDGE descriptors at ~7.4ns/desc on ONE Q7 cpu
pair (cpu_id/2 == queue_num), ~60.7us/gather, and the ring only drains at
gen end (+17us tail).  v1 ran loc+proc gathers per tile (two 60.7us gens)
and adjacent tiles shared a queue, serializing gen->drain->gen: 78.7us/tile.

  - proc part moved OFF gpsimd entirely: one-hot matmuls on the idle PE.
    Host ships, per tile, 8 stationary matrices OHW[G] in [128,128] f32:
    OHW[G][16g+k, p] = (pid[row(p, slot 8G+g)] == k).  rhs is a block-diag
    [128, 64] with proc_pos in 8 diagonal [16,8] blocks, so PSUM picks up
    ps[p, 8s+d] = proc_pos[pid(p,s), d] -- [128, S, 8] contiguous.  Exact
    in fp32 (single 1.0*value product per output).  ACT copies PSUM->outb.
  - loc gather (the only SWDGE user) rotates queues b%4, so up to 4 gens
    run concurrently on the 4 Q7 cpu pairs (Pool exec queue depth = 4) and
    a queue's ring drain overlaps other queues' gens.
  - DEPTH=6 slot ring so enough tiles are in flight to feed 4 queues.

Per-tile pipeline: SP loads eidx/off/ohw + stores out tiles; gpsimd issues
the loc gather (queue b%4); PE runs 8 one-hot matmuls into the tile's PSUM
bank; ACT copies PSUM into outb[:, :, 0:8]; DVE builds the 16 offset masks
and runs 16 copy_predicated extracting 12B/row from the gathered 256B
blocks into outb[:, :, 8:11].
"""

from contextlib import ExitStack

import numpy as np

import concourse.bass as bass
import concourse.mybir as mybir
from concourse import library_config
from concourse.bass_utils import run_bass_kernel_spmd

N_CORES = 8
NAUG = 8_000_000
PER_CORE = NAUG // N_CORES  # 1,000,000

NUM_PROCS = 16
PROC_DIM = 8
NUM_LOCS = 500_000
SPATIAL_DIM = 3
OUT_DIM = PROC_DIM + SPATIAL_DIM  # 11

NBLK = NUM_LOCS // 16  # 31250 blocks of 16 padded rows each

T = 8192  # rows per tile (= indices per dma_gather)
S = T // 128  # slots per partition (64)
NT = -(-PER_CORE // T)  # 123
N_PAD = NT * T  # 1,007,616
DEPTH = 6
NQ = 4  # SWDGE queues


def build_nc():
    nc = bass.Bass(num_swdge_queues=NQ)
    loc64 = nc.declare_dram_parameter("loc64", [NBLK, 64], mybir.dt.float32, isOutput=False)
    pmat_d = nc.declare_dram_parameter("pmat", [128, 64], mybir.dt.float32, isOutput=False)
    iota16 = nc.declare_dram_parameter("iota16", [128, 16], mybir.dt.int32, isOutput=False)
    eidx = nc.declare_dram_parameter("eidx", [NT, 128, T // 16], mybir.dt.int16, isOutput=False)
    ohw = nc.declare_dram_parameter("ohw", [NT, 128, 8 * 128], mybir.dt.float32, isOutput=False)
    off = nc.declare_dram_parameter("off", [NT, 128, S], mybir.dt.int32, isOutput=False)
    out = nc.declare_dram_parameter("out", [N_PAD, OUT_DIM], mybir.dt.float32, isOutput=True)
    out_v = out.rearrange("(b p s) d -> b p (s d)", b=NT, p=128)

    W = T // 16  # wrapped idx columns

    def uses(s):
        return (NT - s + DEPTH - 1) // DEPTH

    with ExitStack() as ctx:
        ldi = ctx.enter_context(nc.semaphore("ldi"))
        ld = [ctx.enter_context(nc.semaphore(f"ld{s}")) for s in range(DEPTH)]
        g = [ctx.enter_context(nc.semaphore(f"g{s}")) for s in range(DEPTH)]
        m = [ctx.enter_context(nc.semaphore(f"m{s}")) for s in range(DEPTH)]
        v = [ctx.enter_context(nc.semaphore(f"v{s}")) for s in range(DEPTH)]
        st = [ctx.enter_context(nc.semaphore(f"st{s}")) for s in range(DEPTH)]
        eidx_sb = ctx.enter_context(nc.sbuf_tensor("eidx_sb", [128, DEPTH * W], mybir.dt.int16))
        ohw_sb = ctx.enter_context(nc.sbuf_tensor("ohw_sb", [128, DEPTH * 8 * 128], mybir.dt.float32))
        off_sb = ctx.enter_context(nc.sbuf_tensor("off_sb", [128, DEPTH * S], mybir.dt.int32))
        iota_sb = ctx.enter_context(nc.sbuf_tensor("iota_sb", [128, 16], mybir.dt.int32))
        pmat_sb = ctx.enter_context(nc.sbuf_tensor("pmat_sb", [128, 64], mybir.dt.float32))
        gloc = ctx.enter_context(nc.sbuf_tensor("gloc", [128, DEPTH * S * 64], mybir.dt.float32))
        outb = ctx.enter_context(nc.sbuf_tensor("outb", [128, DEPTH * S * OUT_DIM], mybir.dt.float32))
        mbuf = ctx.enter_context(nc.sbuf_tensor("mbuf", [128, DEPTH * S * 16], mybir.dt.uint8))
        ps = ctx.enter_context(nc.psum_tensor("ps", [128, DEPTH * S * 8], mybir.dt.float32))
        block = ctx.enter_context(nc.Block())

        def eslot(s):
            return eidx_sb[:, s * W:(s + 1) * W]

        def wslot(s, G):
            return ohw_sb[:, (s * 8 + G) * 128:(s * 8 + G + 1) * 128]

        def oslot(s):
            return off_sb[:, s * S:(s + 1) * S]

        def glslot(s):
            return gloc[:, s * S * 64:(s + 1) * S * 64].rearrange("p (s e) -> p s e", e=64)

        def psslot(s):
            return ps[:, s * S * 8:(s + 1) * S * 8].rearrange("p (s e) -> p s e", e=8)

        def oslot_out(s):
            return outb[:, s * S * OUT_DIM:(s + 1) * S * OUT_DIM].rearrange(
                "p (s e) -> p s e", e=OUT_DIM)

        def mslot(s):
            return mbuf[:, s * S * 16:(s + 1) * S * 16].rearrange("p (s e) -> p s e", e=16)

        def issue_loads(sp, b):
            s = b % DEPTH
            sp.dma_start(out=eslot(s), in_=eidx[b]).then_inc(ld[s], 16)
            sp.dma_start(out=off_sb[:, s * S:(s + 1) * S], in_=off[b]).then_inc(ld[s], 16)
            sp.dma_start(
                out=ohw_sb[:, s * 8 * 128:(s + 1) * 8 * 128], in_=ohw[b]
            ).then_inc(ld[s], 16)

        @block.sync
        def _(sp):
            sp.dma_start(out=iota_sb[:], in_=iota16[:]).then_inc(ldi, 16)
            sp.dma_start(out=pmat_sb[:], in_=pmat_d[:]).then_inc(ldi, 16)
            for b in range(min(DEPTH, NT)):
                issue_loads(sp, b)
            for b in range(NT):
                s, u = b % DEPTH, b // DEPTH
                sp.wait_ge(v[s], 2 * (u + 1))
                sp.dma_start(
                    out=out_v[b],
                    in_=outb[:, s * S * OUT_DIM:(s + 1) * S * OUT_DIM],
                ).then_inc(st[s], 16)
                if b + DEPTH < NT:
                    # slot reuse: eidx free once gather ran; ohw free once PE ran
                    sp.wait_ge(g[s], 16 * (u + 1))
                    sp.wait_ge(m[s], 8 * (u + 1))
                    issue_loads(sp, b + DEPTH)
            for s in range(DEPTH):
                sp.wait_ge(st[s], 16 * uses(s))

        @block.gpsimd
        def _(gp):
            gp.load_library(library_config.mlp)
            t_reg = gp.to_reg(T)
            for b in range(NT):
                s, u = b % DEPTH, b // DEPTH
                gp.wait_ge(ld[s], 48 * (u + 1))
                if u >= 1:
                    # gather dst slot free once the slot's extract+copy ran
                    gp.wait_ge(v[s], 2 * u)
                gp.dma_gather(
                    glslot(s), loc64[:], eslot(s), T, t_reg, 64,
                    queue_num=b % NQ, single_packet=False,
                ).then_inc(g[s], 16)
            for s in range(DEPTH):
                gp.wait_ge(g[s], 16 * uses(s))

        @block.tensor
        def _(pe):
            pe.wait_ge(ldi, 32)
            for b in range(NT):
                s, u = b % DEPTH, b // DEPTH
                pe.wait_ge(ld[s], 48 * (u + 1))
                if u >= 1:
                    # psum slot free once ACT copied the previous use
                    pe.wait_ge(v[s], 2 * u)
                for G in range(8):
                    pe.matmul(
                        ps[:, s * S * 8 + G * 64:s * S * 8 + (G + 1) * 64],
                        wslot(s, G),
                        pmat_sb[:],
                    ).then_inc(m[s], 1)

        @block.scalar
        def _(act):
            for b in range(NT):
                s, u = b % DEPTH, b // DEPTH
                act.wait_ge(m[s], 8 * (u + 1))
                if u >= 1:
                    act.wait_ge(st[s], 16 * u)
                act.copy(out=oslot_out(s)[:, :, 0:PROC_DIM], in_=psslot(s)).then_inc(v[s], 1)

        @block.vector
        def _(dve):
            dve.wait_ge(ldi, 32)
            for b in range(NT):
                s, u = b % DEPTH, b // DEPTH
                dve.wait_ge(ld[s], 48 * (u + 1))
                dve.wait_ge(g[s], 16 * (u + 1))
                if u >= 1:
                    dve.wait_ge(st[s], 16 * u)
                ob = oslot_out(s)
                gl = glslot(s)
                mk = mslot(s)
                dve.tensor_tensor(
                    out=mk,
                    in0=oslot(s)[:, :, None].broadcast_to([128, S, 16]),
                    in1=iota_sb[:, None, :].broadcast_to([128, S, 16]),
                    op=mybir.AluOpType.is_equal,
                )
                dve.drain()
                for o in range(16):
                    ins = dve.copy_predicated(
                        ob[:, :, PROC_DIM:OUT_DIM],
                        mk[:, :, o, None].broadcast_to([128, S, 3]),
                        gl[:, :, 4 * o:4 * o + 3],
                    )
                ins.then_inc(v[s], 1)

    from concourse.library_overlay import lower_extended_insts

    lower_extended_insts(nc)
    return nc


_nc_cache = {}

# test.py reads this for exec_time_ns / trace info after a traced run.
_last_results = None


def _get_nc():
    if "nc" not in _nc_cache:
        _nc_cache["nc"] = build_nc()
    return _nc_cache["nc"]


def _prep_indices(vals, dtype):
    """[NT*T] row-major -> [NT, 128, T//16] wrapped+replicated gather lists.

    Tile row r (= p*S + s) must sit at gather list position j = s*128 + p;
    the wrapped layout stores position j at [j%16, j//16], replicated to all
    8 16-partition groups so every SWDGE queue's q7 pair finds them.
    """
    a = vals.reshape(NT, 128, S)  # [b, p, s]
    a = a.transpose(0, 2, 1).reshape(NT, T // 16, 16)  # [b, j//16, j%16]
    a = a.transpose(0, 2, 1)  # [b, 16, T//16]
    return np.broadcast_to(a[:, None, :, :], (NT, 8, 16, T // 16)).reshape(
        NT, 128, T // 16).astype(dtype)


def kernel(proc_pos, locs_sp, process_ids, location_ids):
    global _last_results
    proc_pos = np.ascontiguousarray(np.asarray(proc_pos, dtype=np.float32))
    locs_sp = np.ascontiguousarray(np.asarray(locs_sp, dtype=np.float32))
    pids = np.asarray(process_ids).astype(np.int32, copy=False)
    lids = np.asarray(location_ids).astype(np.int32, copy=False)

    loc_pad = np.zeros((NBLK * 16, 4), np.float32)
    loc_pad[:NUM_LOCS, :SPATIAL_DIM] = locs_sp
    loc64 = loc_pad.reshape(NBLK, 64)
    # Block-diag rhs: pmat[16g+k, 8g+d] = proc_pos[k, d]
    pmat = np.zeros((128, 64), np.float32)
    for gg in range(8):
        pmat[16 * gg:16 * gg + NUM_PROCS, 8 * gg:8 * gg + PROC_DIM] = proc_pos
    iota16 = np.tile(np.arange(16, dtype=np.int32), (128, 1))

    nc = _get_nc()
    in_maps = []
    for c in range(N_CORES):
        lo, hi = c * PER_CORE, (c + 1) * PER_CORE
        lid_c = np.zeros(N_PAD, np.int32)
        pid_c = np.zeros(N_PAD, np.int32)
        lid_c[:PER_CORE] = lids[lo:hi]
        pid_c[:PER_CORE] = pids[lo:hi]
        # One-hot stationary weights: ohw[b, G, 16g+k, p] = (pid[b,p,8G+g]==k)
        P = pid_c.reshape(NT, 128, 8, 8)  # [b, p, G, g]
        oh = (P[:, :, :, :, None] == np.arange(16, dtype=np.int32)).astype(np.float32)
        # [b, p, G, g, k] -> [b, G, (g,k), p] -> [b, (G,g,k)=8*128, p]... PE wants
        # lhsT partition dim = (g,k), free dim = p: store as [b, 128part=(g,k), 8G*... ]
        # SBUF layout [128, 8*128]: partition = kk=(16g+k), col = G*128 + p.
        ohw_c = np.ascontiguousarray(
            oh.transpose(0, 3, 4, 2, 1)  # [b, g, k, G, p]
            .reshape(NT, 128, 8, 128)    # [b, kk, G, p]
            .reshape(NT, 128, 8 * 128)
        )
        in_maps.append(
            {
                "loc64": loc64,
                "pmat": pmat,
                "iota16": iota16,
                "eidx": _prep_indices(lid_c >> 4, np.int16),
                "ohw": ohw_c,
                "off": (lid_c & 15).astype(np.int32).reshape(NT, 128, S),
            }
        )

    res = run_bass_kernel_spmd(nc, in_maps, list(range(N_CORES)))
    _last_results = res
    out = np.concatenate([r["out"][:PER_CORE] for r in res.results], axis=0)
    return out


# revision 5
# speedup vs baseline: 3.6703x; 1.0465x over previous
"""Trainium2 Bass kernel: fused embedding gather-concat.

out[r] = concat(proc_pos[process_ids[r]], locs_sp[location_ids[r]])   r in [0, 8M)

Sharding: rows data-parallel across 8 NeuronCores (1M rows each, padded to a
tile multiple); lookup tables replicated in every core's DRAM.

v2 (vs v1's 9.68ms): trace showed GPSIMD desc-gen is the wall: each
dma_gather generates 8192 SWDGE descriptors at ~7.4ns/desc on ONE Q7 cpu
pair (cpu_id/2 == queue_num), ~60.7us/gather, and the ring only drains at
gen end (+17us tail).  v1 ran loc+proc gathers per tile (two 60.7us gens)
and adjacent tiles shared a queue, serializing gen->drain->gen: 78.7us/tile.

  - proc part moved OFF gpsimd entirely: one-hot matmuls on the idle PE.
    Host ships, per tile, 8 stationary matrices OHW[G] in [128,128] f32:
    OHW[G][16g+k, p] = (pid[row(p, slot 8G+g)] == k).  rhs is a block-diag
    [128, 64] with proc_pos in 8 diagonal [16,8] blocks, so PSUM picks up
    ps[p, 8s+d] = proc_pos[pid(p,s), d] -- [128, S, 8] contiguous.  Exact
    in fp32 (single 1.0*value product per output).  ACT copies PSUM->outb.
  - loc gather (the only SWDGE user) rotates queues b%4, so up to 4 gens
    run concurrently on the 4 Q7 cpu pairs (Pool exec queue depth = 4) and
    a queue's ring drain overlaps other queues' gens.
  - DEPTH=6 slot ring so enough tiles are in flight to feed 4 queues.

Per-tile pipeline: SP loads eidx/off/ohw + stores out tiles; gpsimd issues
the loc gather (queue b%4); PE runs 8 one-hot matmuls into the tile's PSUM
bank; ACT copies PSUM into outb[:, :, 0:8]; DVE builds the 16 offset masks
and runs 16 copy_predicated extracting 12B/row from the gathered 256B
blocks into outb[:, :, 8:11].
"""

from contextlib import ExitStack

import numpy as np

import concourse.bass as bass
import concourse.mybir as mybir
from concourse import library_config
from concourse.bass_utils import run_bass_kernel_spmd

N_CORES = 8
NAUG = 8_000_000
PER_CORE = NAUG // N_CORES  # 1,000,000

NUM_PROCS = 16
PROC_DIM = 8
NUM_LOCS = 500_000
SPATIAL_DIM = 3
OUT_DIM = PROC_DIM + SPATIAL_DIM  # 11

NBLK = NUM_LOCS // 16  # 31250 blocks of 16 padded rows each

T = 8192  # rows per tile (= indices per dma_gather)
S = T // 128  # slots per partition (64)
NT = -(-PER_CORE // T)  # 123
N_PAD = NT * T  # 1,007,616
DEPTH = 7
NQ = 4  # SWDGE queues


def build_nc():
    nc = bass.Bass(num_swdge_queues=NQ)
    loc64 = nc.declare_dram_parameter("loc64", [NBLK, 64], mybir.dt.float32, isOutput=False)
    pmat_d = nc.declare_dram_parameter("pmat", [128, 64], mybir.dt.float32, isOutput=False)
    iota16 = nc.declare_dram_parameter("iota16", [128, 16], mybir.dt.int32, isOutput=False)
    eidx = nc.declare_dram_parameter("eidx", [NT, 128, T // 16], mybir.dt.int16, isOutput=False)
    ohw = nc.declare_dram_parameter("ohw", [NT, 128, 8 * 128], mybir.dt.float32, isOutput=False)
    off = nc.declare_dram_parameter("off", [NT, 128, S], mybir.dt.int32, isOutput=False)
    out = nc.declare_dram_parameter("out", [N_PAD, OUT_DIM], mybir.dt.float32, isOutput=True)
    out_v = out.rearrange("(b p s) d -> b p (s d)", b=NT, p=128)

    W = T // 16  # wrapped idx columns

    def uses(s):
        return (NT - s + DEPTH - 1) // DEPTH

    with ExitStack() as ctx:
        ldi = ctx.enter_context(nc.semaphore("ldi"))
        ld = [ctx.enter_context(nc.semaphore(f"ld{s}")) for s in range(DEPTH)]
        g = [ctx.enter_context(nc.semaphore(f"g{s}")) for s in range(DEPTH)]
        m = [ctx.enter_context(nc.semaphore(f"m{s}")) for s in range(DEPTH)]
        v = [ctx.enter_context(nc.semaphore(f"v{s}")) for s in range(DEPTH)]
        st = [ctx.enter_context(nc.semaphore(f"st{s}")) for s in range(DEPTH)]
        eidx_sb = ctx.enter_context(nc.sbuf_tensor("eidx_sb", [128, DEPTH * W], mybir.dt.int16))
        ohw_sb = ctx.enter_context(nc.sbuf_tensor("ohw_sb", [128, DEPTH * 8 * 128], mybir.dt.float32))
        off_sb = ctx.enter_context(nc.sbuf_tensor("off_sb", [128, DEPTH * S], mybir.dt.int32))
        iota_sb = ctx.enter_context(nc.sbuf_tensor("iota_sb", [128, 16], mybir.dt.int32))
        pmat_sb = ctx.enter_context(nc.sbuf_tensor("pmat_sb", [128, 64], mybir.dt.float32))
        gloc = ctx.enter_context(nc.sbuf_tensor("gloc", [128, DEPTH * S * 64], mybir.dt.float32))
        outb = ctx.enter_context(nc.sbuf_tensor("outb", [128, DEPTH * S * OUT_DIM], mybir.dt.float32))
        mbuf = ctx.enter_context(nc.sbuf_tensor("mbuf", [128, DEPTH * S * 16], mybir.dt.uint8))
        ps = ctx.enter_context(nc.psum_tensor("ps", [128, DEPTH * S * 8], mybir.dt.float32))
        block = ctx.enter_context(nc.Block())

        def eslot(s):
            return eidx_sb[:, s * W:(s + 1) * W]

        def wslot(s, G):
            return ohw_sb[:, (s * 8 + G) * 128:(s * 8 + G + 1) * 128]

        def oslot(s):
            return off_sb[:, s * S:(s + 1) * S]

        def glslot(s):
            return gloc[:, s * S * 64:(s + 1) * S * 64].rearrange("p (s e) -> p s e", e=64)

        def psslot(s):
            return ps[:, s * S * 8:(s + 1) * S * 8].rearrange("p (s e) -> p s e", e=8)

        def oslot_out(s):
            return outb[:, s * S * OUT_DIM:(s + 1) * S * OUT_DIM].rearrange(
                "p (s e) -> p s e", e=OUT_DIM)

        def mslot(s):
            return mbuf[:, s * S * 16:(s + 1) * S * 16].rearrange("p (s e) -> p s e", e=16)

        def issue_loads(sp, b):
            s = b % DEPTH
            sp.dma_start(out=eslot(s), in_=eidx[b]).then_inc(ld[s], 16)
            sp.dma_start(out=off_sb[:, s * S:(s + 1) * S], in_=off[b]).then_inc(ld[s], 16)
            sp.dma_start(
                out=ohw_sb[:, s * 8 * 128:(s + 1) * 8 * 128], in_=ohw[b]
            ).then_inc(ld[s], 16)

        @block.sync
        def _(sp):
            sp.dma_start(out=iota_sb[:], in_=iota16[:]).then_inc(ldi, 16)
            sp.dma_start(out=pmat_sb[:], in_=pmat_d[:]).then_inc(ldi, 16)
            for b in range(min(DEPTH, NT)):
                issue_loads(sp, b)
            for b in range(NT):
                s, u = b % DEPTH, b // DEPTH
                sp.wait_ge(v[s], 2 * (u + 1))
                sp.dma_start(
                    out=out_v[b],
                    in_=outb[:, s * S * OUT_DIM:(s + 1) * S * OUT_DIM],
                ).then_inc(st[s], 16)
                if b + DEPTH < NT:
                    # slot reuse: eidx free once gather ran; ohw free once PE ran
                    sp.wait_ge(g[s], 16 * (u + 1))
                    sp.wait_ge(m[s], 8 * (u + 1))
                    issue_loads(sp, b + DEPTH)
            for s in range(DEPTH):
                sp.wait_ge(st[s], 16 * uses(s))

        @block.gpsimd
        def _(gp):
            gp.load_library(library_config.mlp)
            t_reg = gp.to_reg(T)
            for b in range(NT):
                s, u = b % DEPTH, b // DEPTH
                gp.wait_ge(ld[s], 48 * (u + 1))
                if u >= 1:
                    # gather dst slot free once the slot's extract+copy ran
                    gp.wait_ge(v[s], 2 * u)
                gp.dma_gather(
                    glslot(s), loc64[:], eslot(s), T, t_reg, 64,
                    queue_num=b % NQ, single_packet=False,
                ).then_inc(g[s], 16)
            for s in range(DEPTH):
                gp.wait_ge(g[s], 16 * uses(s))

        @block.tensor
        def _(pe):
            pe.wait_ge(ldi, 32)
            for b in range(NT):
                s, u = b % DEPTH, b // DEPTH
                pe.wait_ge(ld[s], 48 * (u + 1))
                if u >= 1:
                    # psum slot free once ACT copied the previous use
                    pe.wait_ge(v[s], 2 * u)
                for G in range(8):
                    pe.matmul(
                        ps[:, s * S * 8 + G * 64:s * S * 8 + (G + 1) * 64],
                        wslot(s, G),
                        pmat_sb[:],
                    ).then_inc(m[s], 1)

        @block.scalar
        def _(act):
            for b in range(NT):
                s, u = b % DEPTH, b // DEPTH
                act.wait_ge(m[s], 8 * (u + 1))
                if u >= 1:
                    act.wait_ge(st[s], 16 * u)
                act.copy(out=oslot_out(s)[:, :, 0:PROC_DIM], in_=psslot(s)).then_inc(v[s], 1)

        @block.vector
        def _(dve):
            dve.wait_ge(ldi, 32)

            def build_mask(b):
                # Mask build only needs off + iota (loaded DEPTH tiles ahead).
                # Hoisted a tile early so the ~10us SBUF write-commit latency
                # of the TT's mask happens off the copy_predicated chain's
                # critical path (the drain/first-CP stall seen in traces).
                s, u = b % DEPTH, b // DEPTH
                dve.wait_ge(ld[s], 48 * (u + 1))
                if u >= 1:
                    dve.wait_ge(st[s], 16 * u)  # mbuf/outb slot reusable
                dve.tensor_tensor(
                    out=mslot(s),
                    in0=oslot(s)[:, :, None].broadcast_to([128, S, 16]),
                    in1=iota_sb[:, None, :].broadcast_to([128, S, 16]),
                    op=mybir.AluOpType.is_equal,
                )

            build_mask(0)
            for b in range(NT):
                s, u = b % DEPTH, b // DEPTH
                if b + 1 < NT:
                    build_mask(b + 1)
                dve.wait_ge(g[s], 16 * (u + 1))
                ob = oslot_out(s)
                gl = glslot(s)
                mk = mslot(s)
                for o in range(16):
                    ins = dve.copy_predicated(
                        ob[:, :, PROC_DIM:OUT_DIM],
                        mk[:, :, o, None].broadcast_to([128, S, 3]),
                        gl[:, :, 4 * o:4 * o + 3],
                    )
                ins.then_inc(v[s], 1)

    from concourse.library_overlay import lower_extended_insts

    lower_extended_insts(nc)
    return nc


_nc_cache = {}

# test.py reads this for exec_time_ns / trace info after a traced run.
_last_results = None


def _get_nc():
    if "nc" not in _nc_cache:
        _nc_cache["nc"] = build_nc()
    return _nc_cache["nc"]


def _prep_indices(vals, dtype):
    """[NT*T] row-major -> [NT, 128, T//16] wrapped+replicated gather lists.

    Tile row r (= p*S + s) must sit at gather list position j = s*128 + p;
    the wrapped layout stores position j at [j%16, j//16], replicated to all
    8 16-partition groups so every SWDGE queue's q7 pair finds them.
    """
    a = vals.reshape(NT, 128, S)  # [b, p, s]
    a = a.transpose(0, 2, 1).reshape(NT, T // 16, 16)  # [b, j//16, j%16]
    a = a.transpose(0, 2, 1)  # [b, 16, T//16]
    return np.broadcast_to(a[:, None, :, :], (NT, 8, 16, T // 16)).reshape(
        NT, 128, T // 16).astype(dtype)


def kernel(proc_pos, locs_sp, process_ids, location_ids):
    global _last_results
    proc_pos = np.ascontiguousarray(np.asarray(proc_pos, dtype=np.float32))
    locs_sp = np.ascontiguousarray(np.asarray(locs_sp, dtype=np.float32))
    pids = np.asarray(process_ids).astype(np.int32, copy=False)
    lids = np.asarray(location_ids).astype(np.int32, copy=False)

    loc_pad = np.zeros((NBLK * 16, 4), np.float32)
    loc_pad[:NUM_LOCS, :SPATIAL_DIM] = locs_sp
    loc64 = loc_pad.reshape(NBLK, 64)
    # Block-diag rhs: pmat[16g+k, 8g+d] = proc_pos[k, d]
    pmat = np.zeros((128, 64), np.float32)
    for gg in range(8):
        pmat[16 * gg:16 * gg + NUM_PROCS, 8 * gg:8 * gg + PROC_DIM] = proc_pos
    iota16 = np.tile(np.arange(16, dtype=np.int32), (128, 1))

    nc = _get_nc()
    in_maps = []
    for c in range(N_CORES):
        lo, hi = c * PER_CORE, (c + 1) * PER_CORE
        lid_c = np.zeros(N_PAD, np.int32)
        pid_c = np.zeros(N_PAD, np.int32)
        lid_c[:PER_CORE] = lids[lo:hi]
        pid_c[:PER_CORE] = pids[lo:hi]
        # One-hot stationary weights: ohw[b, G, 16g+k, p] = (pid[b,p,8G+g]==k)
        P = pid_c.reshape(NT, 128, 8, 8)  # [b, p, G, g]
        oh = (P[:, :, :, :, None] == np.arange(16, dtype=np.int32)).astype(np.float32)
        # [b, p, G, g, k] -> [b, G, (g,k), p] -> [b, (G,g,k)=8*128, p]... PE wants
        # lhsT partition dim = (g,k), free dim = p: store as [b, 128part=(g,k), 8G*... ]
        # SBUF layout [128, 8*128]: partition = kk=(16g+k), col = G*128 + p.
        ohw_c = np.ascontiguousarray(
            oh.transpose(0, 3, 4, 2, 1)  # [b, g, k, G, p]
            .reshape(NT, 128, 8, 128)    # [b, kk, G, p]
            .reshape(NT, 128, 8 * 128)
        )
        in_maps.append(
            {
                "loc64": loc64,
                "pmat": pmat,
                "iota16": iota16,
                "eidx": _prep_indices(lid_c >> 4, np.int16),
                "ohw": ohw_c,
                "off": (lid_c & 15).astype(np.int32).reshape(NT, 128, S),
            }
        )

    res = run_bass_kernel_spmd(nc, in_maps, list(range(N_CORES)))
    _last_results = res
    out = np.concatenate([r["out"][:PER_CORE] for r in res.results], axis=0)
    return out
